# revision 47
# baseline (speedup 1.0000x reference)
"""GraphConv GNN kernel for trn2: host preprocessing + bass program builder.

Sharding: nodes (and incident edges, by dst) across 8 cores, owner-major row
layout. Aggregation via per-edge gathered src rows x one-hot matmul
segment-sum on the PE. Weights replicated. One Shared-output AllGather of
node features per layer. Pooled partial sums + head computed per-core,
summed on host.

v2 design, 829us -> ~710us (the baseline was SWDGE-descgen-bound at
~4.9ns/idx on the Q7 cores and DMA-bound on 59MB of one-hot reloads):

- Layer-1 per-edge src rows are pregathered on the HOST (a pure indexing
  transform of the input x, like the existing layout transforms) and
  streamed in as two dense HWDGE reads (lo on the Sync queue, hi on the
  Activation queue). No layer-1 descgen at all.
- One-hot segment-sum matrices are generated ON DEVICE by the (otherwise
  idle) DVE: one broadcast tensor_tensor is_equal per gather-op against a
  resident iota row, from tiny per-chunk slot-id inputs. Kills ~59MB of
  HWDGE traffic (~530ns per [128,CPO,128] op on DVE, fully hidden).
- Layers 2/3 use plain gen_mode=0 dma_gather (ucode descgen+fire) spread
  over the 4 SWDGE queues, emitted AFTER the layer's AllGather so the
  engine head-block waits exactly on the collective. The gather ring
  (GB ops per half) self-paces late ops via the tile WAR on the slot's
  previous readers.
- The staging transpose + DRAM stage for the AllGather is split at the
  window midpoint so the first half stages while the second half still
  computes.

Scheduling facts this kernel is built around (measured on HW):
- dma_gather descgen runs on ONE Q7 core pair selected by queue_num
  (~4.9ns/idx, ~5us per 1024-idx op); 4 queues = 4 concurrent pairs,
  but gpsimd's in-order dispatch + per-op ring WARs cap effective
  concurrency near 2 in the consumption-paced convoy (~225us/layer).
- prepare_only+trigger_dma was attempted to pre-generate descriptors
  across the AllGather and abandoned: Tile releases prep consumers via
  IncSwdgeSem doorbells at DESC-GEN time (not transfer completion), the
  table-read dep is NOT deferred to the trigger for dma_gather (it stays
  on the prep, or is silently absent if the prep is emitted before the
  collective), floating wait_ge instructions get reordered by the
  scheduler (deadlock), and even with waits anchored on the previous
  matmul via wait_op, burst-fired transfers showed progressive staleness
  vs the sem=+16 completion counts on HW (windows 3-25 ~15% stale).
- A 1024-idx gather op = 65 ring descriptors; default dynamic-dma
  scratch gives a 1023-desc ring per queue.
- The AllGather output must be a Shared DRAM tensor (single writer) for
  the fast CC path; chunked collectives pay ~15-25us fixed each and
  lose. AG costs ~52us/layer incl. trailing CC phases.
- Tile's HWDGE DMA semaphore lanes are cumulative in program order, so
  HWDGE DMAs are emitted in consumption order per engine queue.
"""

import sys

sys.path.insert(0, "/opt/trn_rl_repo")

import numpy as np
import ml_dtypes

import concourse.bass as bass
import concourse.bacc as bacc
import concourse.tile as tile
import concourse.mybir as mybir
from concourse import library_config

BF16 = mybir.dt.bfloat16
F32 = mybir.dt.float32
I16 = mybir.dt.int16

N_CORES = 8
F = 128
N_CLASSES = 10

# per-window structure: K_LO lo-chunks + K_HI hi-chunks of 128 edges each
K_LO = 6
K_HI = 6
EDGES_PER_HALF = K_LO * 128  # 768
CHUNKS_PER_WIN = K_LO + K_HI

CPO = 8  # chunks (of 128 edges) per gather op
GB = 16  # gather-tile ring depth in ops, per half
SB = 8   # s-tile ring depth in ops, per half


def _wrap_idx(idx_flat):
    """idx i -> partition i%16, col i//16; replicated across the 8 Q7 core
    stripes (16 partitions each)."""
    n = idx_flat.shape[0]
    return np.ascontiguousarray(
        np.tile(idx_flat.reshape(n // 16, 16).T.astype(np.int16), (8, 1))
    )


def preprocess(x, edge_index, batch, params, n_nodes, n_graphs):
    """Build per-core inputs + meta for the SPMD program."""
    assert n_nodes % N_CORES == 0
    npc = n_nodes // N_CORES
    src = np.asarray(edge_index[0], np.int64)
    dst = np.asarray(edge_index[1], np.int64)
    batch = np.asarray(batch, np.int64)
    x = np.asarray(x, np.float32)
    x_bf = x.astype(ml_dtypes.bfloat16)

    # owner-based lo/hi split: rows are owner-major (single AllGather
    # concatenates whole per-core shards), so lo = cores 0-3
    half_node = (N_CORES // 2) * npc
    is_lo_node = np.arange(n_nodes) < half_node

    # sort edges by dst once
    order = np.argsort(dst, kind="stable")
    src_s, dst_s = src[order], dst[order]

    # per-core edge ranges
    core_edge_start = np.searchsorted(dst_s, np.arange(0, n_nodes + 1, npc))

    # --- pass 1: greedy windows per core ---
    core_windows = []
    for k in range(N_CORES):
        e0, e1 = core_edge_start[k], core_edge_start[k + 1]
        dl = dst_s[e0:e1] - k * npc
        sl_lo = is_lo_node[src_s[e0:e1]]
        deg_lo = np.bincount(dl[sl_lo], minlength=npc)
        deg_hi = np.bincount(dl[~sl_lo], minlength=npc)
        wins = []
        d = 0
        while d < npc:
            start = d
            lo = hi = 0
            while (
                d < npc
                and d - start < 128
                and lo + deg_lo[d] <= EDGES_PER_HALF
                and hi + deg_hi[d] <= EDGES_PER_HALF
            ):
                lo += deg_lo[d]
                hi += deg_hi[d]
                d += 1
            assert d > start, "single dst exceeds per-window edge budget"
            wins.append((start, d))
        core_windows.append(wins)

    w_star = max(len(w) for w in core_windows)
    mid = (w_star + 1) // 2
    ls = w_star * 128
    rows = N_CORES * ls
    half_rows = rows // 2
    assert half_rows <= 32767 and rows - half_rows <= 32767
    for k in range(N_CORES):
        core_windows[k] = core_windows[k] + [(npc, npc)] * (
            w_star - len(core_windows[k])
        )

    # --- slots + rows for every node ---
    slot = np.full(n_nodes, -1, np.int64)
    for k in range(N_CORES):
        for c, (a, b) in enumerate(core_windows[k]):
            if b > a:
                d_loc = np.arange(a, b)
                slot[k * npc + d_loc] = c * 128 + (d_loc - a)
    assert (slot >= 0).all()
    owner = np.arange(n_nodes) // npc
    c_of = slot // 128
    p_of = slot % 128
    # owner-major rows: transpose convention fm pos s -> (p=s%128, c=s//128)
    row_of = owner * ls + p_of * w_star + c_of

    n_chunks = w_star * K_LO  # chunks per half
    n_ops = -(-n_chunks // CPO)

    # --- per-core streams ---
    per_core = []
    for k in range(N_CORES):
        e0, e1 = core_edge_start[k], core_edge_start[k + 1]
        dl = dst_s[e0:e1] - k * npc
        sv = src_s[e0:e1]
        is_lo = is_lo_node[sv]
        idx_lo = np.zeros((w_star, EDGES_PER_HALF), np.int64)
        ids_lo = np.full((w_star, EDGES_PER_HALF), -1.0, np.float32)
        nd_lo = np.zeros((w_star, EDGES_PER_HALF), np.int64)  # src node ids
        idx_hi = np.zeros_like(idx_lo)
        ids_hi = np.full_like(ids_lo, -1.0)
        nd_hi = np.zeros_like(nd_lo)
        # edges are dst-sorted; window edge groups are contiguous
        wbounds = np.searchsorted(dl, [a for a, _ in core_windows[k]] + [npc])
        for w, (a, b) in enumerate(core_windows[k]):
            lo_m = is_lo[wbounds[w] : wbounds[w + 1]]
            e_dst = dl[wbounds[w] : wbounds[w + 1]]
            e_src = sv[wbounds[w] : wbounds[w + 1]]
            for half, m in ((0, lo_m), (1, ~lo_m)):
                r = row_of[e_src[m]] - (0 if half == 0 else half_rows)
                cnt = r.shape[0]
                assert cnt <= EDGES_PER_HALF
                tgt_idx = idx_lo if half == 0 else idx_hi
                tgt_ids = ids_lo if half == 0 else ids_hi
                tgt_nd = nd_lo if half == 0 else nd_hi
                tgt_idx[w, :cnt] = r
                tgt_ids[w, :cnt] = (e_dst[m] - a).astype(np.float32)
                tgt_nd[w, :cnt] = e_src[m]

        def _ids_wrap(ids_arr):
            # [n_chunks, 128] -> [128, n_chunks] bf16
            return np.ascontiguousarray(
                ids_arr.reshape(n_chunks, 128).T.astype(ml_dtypes.bfloat16)
            )

        def _xg(nd_arr):
            # pregathered layer-1 rows: [128, n_chunks * F] bf16
            g = x_bf[nd_arr.reshape(n_chunks, 128)]  # [nch, 128, F]
            return np.ascontiguousarray(
                g.transpose(1, 0, 2).reshape(128, n_chunks * F)
            )

        per_core.append(
            dict(
                idx_lo=_wrap_idx(idx_lo.reshape(-1)),
                idx_hi=_wrap_idx(idx_hi.reshape(-1)),
                ids_lo=_ids_wrap(ids_lo),
                ids_hi=_ids_wrap(ids_hi),
                xg_lo=_xg(nd_lo),
                xg_hi=_xg(nd_hi),
            )
        )

    iota_np = np.ascontiguousarray(
        np.tile(np.arange(128, dtype=np.float32), (128, 1)).astype(
            ml_dtypes.bfloat16
        )
    )

    in_maps = []
    for k in range(N_CORES):
        g = np.arange(k * npc, (k + 1) * npc)
        x_fm = np.zeros((F, ls), ml_dtypes.bfloat16)
        x_fm[:, slot[g]] = x_bf[g].T
        b_flat = np.full(ls, -1.0, np.float32)
        b_flat[slot[g]] = batch[g].astype(np.float32)
        batch_nm = b_flat.reshape(w_star, 128).T  # [p, c]
        b_onehot = (
            batch_nm[:, :, None] == np.arange(64, dtype=np.float32)[None, None, :]
        )
        b_onehot = np.ascontiguousarray(
            b_onehot.reshape(128, w_star * 64).astype(ml_dtypes.bfloat16)
        )
        m = dict(
            x_fm=x_fm,
            b_onehot=b_onehot,
            iota=iota_np,
            idx_lo=per_core[k]["idx_lo"],
            idx_hi=per_core[k]["idx_hi"],
            ids_lo=per_core[k]["ids_lo"],
            ids_hi=per_core[k]["ids_hi"],
            xg_lo=per_core[k]["xg_lo"],
            xg_hi=per_core[k]["xg_hi"],
            w1relT=np.ascontiguousarray(params["W1_rel"].T.astype(ml_dtypes.bfloat16)),
            w1rootT=np.ascontiguousarray(
                params["W1_root"].T.astype(ml_dtypes.bfloat16)
            ),
            w2relT=np.ascontiguousarray(params["W2_rel"].T.astype(ml_dtypes.bfloat16)),
            w2rootT=np.ascontiguousarray(
                params["W2_root"].T.astype(ml_dtypes.bfloat16)
            ),
            w3relT=np.ascontiguousarray(params["W3_rel"].T.astype(ml_dtypes.bfloat16)),
            w3rootT=np.ascontiguousarray(
                params["W3_root"].T.astype(ml_dtypes.bfloat16)
            ),
            b1=np.ascontiguousarray(params["b1_rel"].astype(np.float32).reshape(F, 1)),
            b2=np.ascontiguousarray(params["b2_rel"].astype(np.float32).reshape(F, 1)),
            b3=np.ascontiguousarray(params["b3_rel"].astype(np.float32).reshape(F, 1)),
            wlinT=np.ascontiguousarray(params["W_lin"].T.astype(np.float32)),
        )
        in_maps.append(m)

    meta = dict(
        w_star=w_star,
        ls=ls,
        rows=rows,
        half_rows=half_rows,
        n_graphs=n_graphs,
        mid=mid,
        n_chunks=n_chunks,
        n_ops=n_ops,
    )
    return meta, in_maps


def build_nc(meta, n_graphs_pad=64, debug_dumps=False):
    w_star = meta["w_star"]
    ls = meta["ls"]
    rows = meta["rows"]
    half_rows = meta["half_rows"]
    mid = meta["mid"]
    n_chunks = meta["n_chunks"]
    n_ops = meta["n_ops"]
    ng = n_graphs_pad
    gb = min(GB, n_ops)
    halves = ("lo", "hi")

    def nch_of(o):
        return min(n_chunks, (o + 1) * CPO) - o * CPO

    def q_of(h, o):
        return (o % 2) if h == "lo" else 2 + (o % 2)

    nc = bacc.Bacc(
        "TRN2",
        target_bir_lowering=False,
        debug=False,
        num_devices=N_CORES,
        num_swdge_queues=4,
    )

    # --- I/O ---
    x_fm_d = nc.dram_tensor("x_fm", [F, ls], BF16, kind="ExternalInput")
    bone_d = nc.dram_tensor("b_onehot", [128, w_star * 64], BF16, kind="ExternalInput")
    iota_d = nc.dram_tensor("iota", [128, 128], BF16, kind="ExternalInput")
    idx_d = {
        h: nc.dram_tensor(
            f"idx_{h}", [128, n_chunks * 8], I16, kind="ExternalInput"
        )
        for h in halves
    }
    ids_d = {
        h: nc.dram_tensor(f"ids_{h}", [128, n_chunks], BF16, kind="ExternalInput")
        for h in halves
    }
    xg_d = {
        h: nc.dram_tensor(
            f"xg_{h}", [128, n_chunks * F], BF16, kind="ExternalInput"
        )
        for h in halves
    }
    w_d = {}
    for l in (1, 2, 3):
        for p in ("rel", "root"):
            w_d[l, p] = nc.dram_tensor(f"w{l}{p}T", [F, F], BF16, kind="ExternalInput")
    b_d = {l: nc.dram_tensor(f"b{l}", [F, 1], F32, kind="ExternalInput") for l in (1, 2, 3)}
    wlin_d = nc.dram_tensor("wlinT", [F, N_CLASSES], F32, kind="ExternalInput")
    out_d = nc.dram_tensor("out_partial", [N_CLASSES, ng], F32, kind="ExternalOutput")
    dump_d = {}
    if debug_dumps:
        for l in (1, 2, 3):
            dump_d[l] = nc.dram_tensor(f"h_dump_{l}", [F, ls], BF16,
                                       kind="ExternalOutput")
        dump_d["agg1"] = nc.dram_tensor("agg_dump_1", [F, ls], BF16,
                                        kind="ExternalOutput")
        dump_d["g"] = nc.dram_tensor("g_dump", [128, CPO * F], BF16,
                                     kind="ExternalOutput")
        dump_d["hf1"] = nc.dram_tensor("hf1_dump", [rows, F], BF16,
                                       kind="ExternalOutput")

    relu = mybir.ActivationFunctionType.Relu
    ident = mybir.ActivationFunctionType.Identity
    copy_f = mybir.ActivationFunctionType.Copy

    with tile.TileContext(nc) as tc:
        with (
            tc.tile_pool(name="const", bufs=1) as constp,
            tc.tile_pool(name="state", bufs=1) as statep,
            tc.tile_pool(name="gpool", bufs=gb) as gpool,
            tc.tile_pool(name="spool", bufs=SB) as spool,
            tc.tile_pool(name="psa", bufs=4, space="PSUM") as psa,
            tc.tile_pool(name="psd", bufs=2, space="PSUM") as psd,
            tc.tile_pool(name="psp", bufs=1, space="PSUM") as psp,
            tc.tile_pool(name="dram", bufs=1, space="DRAM") as dramp,
        ):
            nc.gpsimd.load_library(library_config.mlp)
            # NOTE on prepare_only/trigger_dma: attempted, abandoned. Tile's
            # consumer sync for gen_mode=1 preps releases at desc-gen time
            # (IncSwdgeSem doorbells), and explicit consumer gating on the
            # prep's sem= still showed progressive staleness during burst
            # fires on HW (completion increments lead the data). gen_mode=0
            # gathers have correct Tile-managed completion semantics.

            # ---- constants (sync HWDGE lane, consumption order) ----
            idx_t = {}
            for h in halves:
                it = constp.tile([128, n_chunks * 8], I16, name=f"idx_{h}")
                nc.sync.dma_start(it[:], idx_d[h][:])
                idx_t[h] = it
            iota_t = constp.tile([128, 128], BF16)
            nc.sync.dma_start(iota_t[:], iota_d[:])
            ids_t = {}
            for h in halves:
                st = constp.tile([128, n_chunks], BF16, name=f"ids_{h}")
                nc.sync.dma_start(st[:], ids_d[h][:])
                ids_t[h] = st
            w_t = {}
            for key, d in w_d.items():
                wt = constp.tile([F, F], BF16, name=f"w_{key[0]}_{key[1]}")
                nc.sync.dma_start(wt[:], d[:])
                w_t[key] = wt
            b_t = {}
            for l, d in b_d.items():
                bt = constp.tile([F, 1], F32, name=f"b_{l}")
                nc.sync.dma_start(bt[:], d[:])
                b_t[l] = bt
            wlin_t = constp.tile([F, N_CLASSES], F32)
            nc.sync.dma_start(wlin_t[:], wlin_d[:])
            x_fm_t = statep.tile([F, ls], BF16, tag="h0")
            nc.sync.dma_start(x_fm_t[:], x_fm_d[:])
            # bone rides the scalar HWDGE lane, needed only at pooling
            bone_t = constp.tile([128, w_star * 64], BF16)
            nc.scalar.dma_start(bone_t[:], bone_d[:])

            # ---- per-layer state ----
            h_fm = x_fm_t
            hf = {}       # AllGather outputs per layer
            ag_in = {}
            for layer in (1, 2):
                ag_in[layer] = dramp.tile(
                    [128, ls], BF16, name=f"agin_{layer}", tag=f"agin{layer % 2}"
                )
                hf[layer] = dramp.tile(
                    [rows, F], BF16, name=f"hf_{layer}", tag=f"hf{layer}",
                    addr_space="Shared",
                )

            def gather_src(layer, h):
                tbl = hf[layer - 1]
                return tbl[0:half_rows, :] if h == "lo" else tbl[half_rows:rows, :]

            def emit_sgen(layer, s_tiles, o):
                c0 = o * CPO
                nch = nch_of(o)
                for h in halves:
                    st_ = spool.tile(
                        [128, nch, 128], BF16,
                        name=f"s_{layer}_{h}_{o}", tag=f"s_{h}",
                        padded_shape=[128, CPO, 128],
                    )
                    in0 = iota_t[:].unsqueeze(1).broadcast_to([128, nch, 128])
                    in1 = ids_t[h][:, c0 : c0 + nch].unsqueeze(2).broadcast_to(
                        [128, nch, 128]
                    )
                    nc.vector.tensor_tensor(
                        st_[:], in0, in1, mybir.AluOpType.is_equal
                    )
                    s_tiles[h][o] = st_

            def alloc_gtile(layer, h, o):
                nch = nch_of(o)
                return gpool.tile(
                    [128, nch, F], BF16,
                    name=f"g_{layer}_{h}_{o}", tag=f"g_{h}",
                    padded_shape=[128, CPO, F],
                )

            def emit_l1_load(g_tiles, o):
                nch = nch_of(o)
                c0 = o * CPO
                for h, eng in (("lo", nc.sync), ("hi", nc.scalar)):
                    gt = alloc_gtile(1, h, o)
                    eng.dma_start(
                        gt[:].rearrange("p c f -> p (c f)"),
                        xg_d[h][:, c0 * F : (c0 + nch) * F],
                    )
                    g_tiles[h][o] = gt

            def emit_gather(layer, g_tiles, o):
                nch = nch_of(o)
                c0 = o * CPO
                nidx = nch * 128
                for h in halves:
                    gt = alloc_gtile(layer, h, o)
                    nc.gpsimd.dma_gather(
                        gt[:],
                        gather_src(layer, h),
                        idx_t[h][:, c0 * 8 : (c0 + nch) * 8],
                        nidx,
                        nidx,
                        F,
                        single_packet=True,
                        queue_num=q_of(h, o),
                    )
                    g_tiles[h][o] = gt

            def emit_layer_compute(layer, g_tiles, s_tiles, fence_src):
                """Emit windows + dense + stage for `layer`.

                fence_src: SBUF tile loaded from hf post-collective (layer>=2)
                anchoring the fence matmul that carries the first gather-op's
                completion wait in the PE stream.
                """
                agg_fm = statep.tile([F, ls], BF16, tag="agg", name=f"agg_{layer}")
                h_next = statep.tile(
                    [F, ls], BF16, tag=f"h{layer % 2}", name=f"h_{layer}"
                )
                h_nm = statep.tile(
                    [128, w_star, F], BF16, tag="hnm", name=f"hnm_{layer}"
                )

                # emission pointers for slot-release-paced late work
                next_sgen = [min(SB, n_ops)]
                next_gop = [gb]

                def release(w):
                    # s-gen for op o allowed once window consuming op o-SB done
                    while next_sgen[0] < n_ops and (
                        ((next_sgen[0] - SB) * CPO + CPO - 1) // K_LO <= w
                    ):
                        emit_sgen(layer, s_tiles, next_sgen[0])
                        next_sgen[0] += 1
                    while next_gop[0] < n_ops and (
                        ((next_gop[0] - gb) * CPO + CPO - 1) // K_LO <= w
                    ):
                        o = next_gop[0]
                        if layer == 1:
                            emit_l1_load(g_tiles, o)
                        else:
                            emit_gather(layer, g_tiles, o)
                        next_gop[0] += 1

                def dense_block(c0_, cq1):
                    while c0_ < cq1:
                        cw = min(512, cq1 - c0_)
                        ps = psd.tile(
                            [128, 512], F32, name=f"psd_{layer}_{c0_}", tag="psd"
                        )
                        sl2 = slice(c0_, c0_ + cw)
                        nc.tensor.matmul(
                            ps[:, :cw], w_t[layer, "rel"][:], agg_fm[:, sl2],
                            start=True, stop=False,
                        )
                        nc.tensor.matmul(
                            ps[:, :cw], w_t[layer, "root"][:], h_fm[:, sl2],
                            start=False, stop=True,
                        )
                        nc.scalar.activation(
                            h_next[:, sl2], ps[:, :cw],
                            relu if layer < 3 else ident,
                            bias=b_t[layer][:],
                        )
                        c0_ += cw

                def stage(a, b):
                    nc.sync.dma_start_transpose(
                        h_nm[:, a:b, :], h_next[:, a * 128 : b * 128]
                    )
                    if layer < 3:
                        nc.sync.dma_start(
                            ag_in[layer][:, a * F : b * F], h_nm[:, a:b, :]
                        )

                for w in range(w_star):
                    ps = psa.tile(
                        [128, 128], F32, name=f"psagg_{layer}_{w}", tag="psagg"
                    )
                    for j in range(CHUNKS_PER_WIN):
                        h = "lo" if j < K_LO else "hi"
                        cc = w * K_LO + (j % K_LO)
                        o, sl_ = cc // CPO, cc % CPO
                        nc.tensor.matmul(
                            ps[:],
                            g_tiles[h][o][:, sl_, :],
                            s_tiles[h][o][:, sl_, :],
                            start=(j == 0),
                            stop=(j == CHUNKS_PER_WIN - 1),
                        )
                    nc.scalar.activation(
                        agg_fm[:, w * 128 : (w + 1) * 128], ps[:], copy_f
                    )
                    if debug_dumps and layer == 2 and w == 0:
                        nc.sync.dma_start(
                            dump_d["g"][:],
                            g_tiles["lo"][0][:].rearrange("p c f -> p (c f)"),
                        )
                    release(w)
                    if w == mid - 1:
                        dense_block(0, mid * 128)
                        stage(0, mid)
                dense_block(mid * 128, ls)
                stage(mid, w_star)
                if debug_dumps:
                    nc.sync.dma_start(dump_d[layer][:], h_next[:])
                    if layer == 1:
                        nc.sync.dma_start(dump_d["agg1"][:], agg_fm[:])
                return h_next, h_nm

            # ================= emission =================
            # Layer 1: initial s-gen + loads
            s_tiles = {1: {h: [None] * n_ops for h in halves}}
            g_tiles = {1: {h: [None] * n_ops for h in halves}}
            for o in range(min(SB, n_ops)):
                emit_sgen(1, s_tiles[1], o)
            for o in range(gb):
                emit_l1_load(g_tiles[1], o)

            h1, _ = emit_layer_compute(1, g_tiles[1], s_tiles[1], None)

            # AG1, then layer-2 gathers (each waits the collective via the
            # Tile-tracked hf data dep; first gather head-blocks gpsimd
            # until AG1 completes, which is why AG1 is emitted first)
            s_tiles[2] = {h: [None] * n_ops for h in halves}
            g_tiles[2] = {h: [None] * n_ops for h in halves}
            nc.gpsimd.collective_compute(
                "AllGather",
                mybir.AluOpType.bypass,
                replica_groups=[list(range(N_CORES))],
                ins=[ag_in[1][:]],
                outs=[hf[1][:]],
            )
            if debug_dumps:
                nc.sync.dma_start(dump_d["hf1"][:], hf[1][:])
            for o in range(gb):
                emit_gather(2, g_tiles[2], o)

            # Layer 2 s-gen head + compute
            h_fm = h1
            for o in range(min(SB, n_ops)):
                emit_sgen(2, s_tiles[2], o)
            h2, _ = emit_layer_compute(2, g_tiles[2], s_tiles[2], None)

            # AG2, then layer-3 gathers
            s_tiles[3] = {h: [None] * n_ops for h in halves}
            g_tiles[3] = {h: [None] * n_ops for h in halves}
            nc.gpsimd.collective_compute(
                "AllGather",
                mybir.AluOpType.bypass,
                replica_groups=[list(range(N_CORES))],
                ins=[ag_in[2][:]],
                outs=[hf[2][:]],
            )
            for o in range(gb):
                emit_gather(3, g_tiles[3], o)

            # Layer 3 s-gen head + compute
            h_fm = h2
            for o in range(min(SB, n_ops)):
                emit_sgen(3, s_tiles[3], o)
            _, h_nm3 = emit_layer_compute(3, g_tiles[3], s_tiles[3], None)

            # pooling: pooledT[f, g] += h_nm[:, c, :].T @ B
            ps_pool = psp.tile([128, ng], F32, tag="pspool")
            for c in range(w_star):
                nc.tensor.matmul(
                    ps_pool[:],
                    h_nm3[:, c, :],
                    bone_t[:, c * ng : (c + 1) * ng],
                    start=(c == 0),
                    stop=(c == w_star - 1),
                )
            pooledT = statep.tile([128, ng], F32, tag="pooledT")
            nc.scalar.activation(pooledT[:], ps_pool[:], copy_f)
            ps_head = psp.tile([N_CLASSES, ng], F32, tag="pshead")
            nc.tensor.matmul(ps_head[:], wlin_t[:], pooledT[:])
            out_sb = statep.tile([N_CLASSES, ng], F32, tag="outsb")
            nc.vector.tensor_copy(out_sb[:], ps_head[:])
            nc.sync.dma_start(out_d[:], out_sb[:])

    nc.compile()
    return nc


def postprocess(results, batch, b_lin, n_graphs):
    """results: list of per-core dicts with 'out_partial' [10, ng]."""
    total = np.zeros_like(np.asarray(results[0]["out_partial"], np.float32))
    for r in results:
        total += np.asarray(r["out_partial"], np.float32)
    cnt = np.bincount(np.asarray(batch, np.int64), minlength=n_graphs).astype(
        np.float32
    )
    cnt = np.maximum(cnt, 1.0)
    logits = total[:, :n_graphs].T / cnt[:, None] + np.asarray(b_lin, np.float32)[None, :]
    return logits.astype(np.float32)


# ----------------------------------------------------------------------------
# harness entry point
# ----------------------------------------------------------------------------
from concourse.bass_utils import run_bass_kernel_spmd

_CACHE = {}


def kernel(x, edge_index, batch,
           W1_rel, b1_rel, W1_root,
           W2_rel, b2_rel, W2_root,
           W3_rel, b3_rel, W3_root,
           W_lin, b_lin):
    params = dict(W1_rel=W1_rel, b1_rel=b1_rel, W1_root=W1_root,
                  W2_rel=W2_rel, b2_rel=b2_rel, W2_root=W2_root,
                  W3_rel=W3_rel, b3_rel=b3_rel, W3_root=W3_root,
                  W_lin=W_lin, b_lin=b_lin)
    n_nodes = int(np.asarray(x).shape[0])
    n_graphs = 64
    meta, in_maps = preprocess(x, edge_index, batch, params, n_nodes, n_graphs)
    key = (meta["w_star"], meta["n_ops"])
    if key not in _CACHE:
        _CACHE[key] = build_nc(meta)
    nc = _CACHE[key]
    res = run_bass_kernel_spmd(nc, in_maps, core_ids=list(range(N_CORES)))
    return postprocess(res.results, batch, b_lin, n_graphs)


# revision 52
# speedup vs baseline: 1.0013x; 1.0013x over previous
"""GraphConv GNN kernel for trn2: host preprocessing + bass program builder.

Sharding: nodes (and incident edges, by dst) across 8 cores, owner-major row
layout. Aggregation via per-edge gathered src rows x one-hot matmul
segment-sum on the PE. Weights replicated. One Shared-output AllGather of
node features per layer. Pooled partial sums + head computed per-core,
summed on host.

v2 design, 829us -> ~710us (the baseline was SWDGE-descgen-bound at
~4.9ns/idx on the Q7 cores and DMA-bound on 59MB of one-hot reloads):

- Layer-1 per-edge src rows are pregathered on the HOST (a pure indexing
  transform of the input x, like the existing layout transforms) and
  streamed in as two dense HWDGE reads (lo on the Sync queue, hi on the
  Activation queue). No layer-1 descgen at all.
- One-hot segment-sum matrices are generated ON DEVICE by the (otherwise
  idle) DVE: one broadcast tensor_tensor is_equal per gather-op against a
  resident iota row, from tiny per-chunk slot-id inputs. Kills ~59MB of
  HWDGE traffic (~530ns per [128,CPO,128] op on DVE, fully hidden).
- Layers 2/3 use plain gen_mode=0 dma_gather (ucode descgen+fire) spread
  over the 4 SWDGE queues, emitted AFTER the layer's AllGather so the
  engine head-block waits exactly on the collective. The gather ring
  (GB ops per half) self-paces late ops via the tile WAR on the slot's
  previous readers.
- The staging transpose + DRAM stage for the AllGather is split at the
  window midpoint so the first half stages while the second half still
  computes.

Scheduling facts this kernel is built around (measured on HW):
- dma_gather descgen runs on ONE Q7 core pair selected by queue_num
  (~4.9ns/idx, ~5us per 1024-idx op); 4 queues = 4 concurrent pairs,
  but gpsimd's in-order dispatch + per-op ring WARs cap effective
  concurrency near 2 in the consumption-paced convoy (~225us/layer).
- prepare_only+trigger_dma was attempted to pre-generate descriptors
  across the AllGather and abandoned: Tile releases prep consumers via
  IncSwdgeSem doorbells at DESC-GEN time (not transfer completion), the
  table-read dep is NOT deferred to the trigger for dma_gather (it stays
  on the prep, or is silently absent if the prep is emitted before the
  collective), floating wait_ge instructions get reordered by the
  scheduler (deadlock), and even with waits anchored on the previous
  matmul via wait_op, burst-fired transfers showed progressive staleness
  vs the sem=+16 completion counts on HW (windows 3-25 ~15% stale).
- A 1024-idx gather op = 65 ring descriptors; default dynamic-dma
  scratch gives a 1023-desc ring per queue.
- The AllGather output must be a Shared DRAM tensor (single writer) for
  the fast CC path; chunked collectives pay ~15-25us fixed each and
  lose. AG costs ~52us/layer incl. trailing CC phases.
- Tile's HWDGE DMA semaphore lanes are cumulative in program order, so
  HWDGE DMAs are emitted in consumption order per engine queue.
"""

import sys

sys.path.insert(0, "/opt/trn_rl_repo")

import numpy as np
import ml_dtypes

import concourse.bass as bass
import concourse.bacc as bacc
import concourse.tile as tile
import concourse.mybir as mybir
from concourse import library_config

BF16 = mybir.dt.bfloat16
F32 = mybir.dt.float32
I16 = mybir.dt.int16

N_CORES = 8
F = 128
N_CLASSES = 10

# per-window structure: K_LO lo-chunks + K_HI hi-chunks of 128 edges each
K_LO = 6
K_HI = 6
EDGES_PER_HALF = K_LO * 128  # 768
CHUNKS_PER_WIN = K_LO + K_HI

CPO = 8  # chunks (of 128 edges) per gather op (ucode crashes at 16)
GB = 16  # gather-tile ring depth in ops, per half
SB = 8   # s-tile ring depth in ops, per half


def _wrap_idx(idx_flat):
    """idx i -> partition i%16, col i//16; replicated across the 8 Q7 core
    stripes (16 partitions each)."""
    n = idx_flat.shape[0]
    return np.ascontiguousarray(
        np.tile(idx_flat.reshape(n // 16, 16).T.astype(np.int16), (8, 1))
    )


def preprocess(x, edge_index, batch, params, n_nodes, n_graphs):
    """Build per-core inputs + meta for the SPMD program."""
    assert n_nodes % N_CORES == 0
    npc = n_nodes // N_CORES
    src = np.asarray(edge_index[0], np.int64)
    dst = np.asarray(edge_index[1], np.int64)
    batch = np.asarray(batch, np.int64)
    x = np.asarray(x, np.float32)
    x_bf = x.astype(ml_dtypes.bfloat16)

    # owner-based lo/hi split: rows are owner-major (single AllGather
    # concatenates whole per-core shards), so lo = cores 0-3
    half_node = (N_CORES // 2) * npc
    is_lo_node = np.arange(n_nodes) < half_node

    # sort edges by dst once
    order = np.argsort(dst, kind="stable")
    src_s, dst_s = src[order], dst[order]

    # per-core edge ranges
    core_edge_start = np.searchsorted(dst_s, np.arange(0, n_nodes + 1, npc))

    # --- pass 1: greedy windows per core ---
    core_windows = []
    for k in range(N_CORES):
        e0, e1 = core_edge_start[k], core_edge_start[k + 1]
        dl = dst_s[e0:e1] - k * npc
        sl_lo = is_lo_node[src_s[e0:e1]]
        deg_lo = np.bincount(dl[sl_lo], minlength=npc)
        deg_hi = np.bincount(dl[~sl_lo], minlength=npc)
        wins = []
        d = 0
        while d < npc:
            start = d
            lo = hi = 0
            while (
                d < npc
                and d - start < 128
                and lo + deg_lo[d] <= EDGES_PER_HALF
                and hi + deg_hi[d] <= EDGES_PER_HALF
            ):
                lo += deg_lo[d]
                hi += deg_hi[d]
                d += 1
            assert d > start, "single dst exceeds per-window edge budget"
            wins.append((start, d))
        core_windows.append(wins)

    w_star = max(len(w) for w in core_windows)
    mid = (w_star + 1) // 2
    ls = w_star * 128
    rows = N_CORES * ls
    half_rows = rows // 2
    assert half_rows <= 32767 and rows - half_rows <= 32767
    for k in range(N_CORES):
        core_windows[k] = core_windows[k] + [(npc, npc)] * (
            w_star - len(core_windows[k])
        )

    # --- slots + rows for every node ---
    slot = np.full(n_nodes, -1, np.int64)
    for k in range(N_CORES):
        for c, (a, b) in enumerate(core_windows[k]):
            if b > a:
                d_loc = np.arange(a, b)
                slot[k * npc + d_loc] = c * 128 + (d_loc - a)
    assert (slot >= 0).all()
    owner = np.arange(n_nodes) // npc
    c_of = slot // 128
    p_of = slot % 128
    # owner-major rows: transpose convention fm pos s -> (p=s%128, c=s//128)
    row_of = owner * ls + p_of * w_star + c_of

    n_chunks = w_star * K_LO  # chunks per half
    n_ops = -(-n_chunks // CPO)

    # --- per-core streams ---
    per_core = []
    for k in range(N_CORES):
        e0, e1 = core_edge_start[k], core_edge_start[k + 1]
        dl = dst_s[e0:e1] - k * npc
        sv = src_s[e0:e1]
        is_lo = is_lo_node[sv]
        idx_lo = np.zeros((w_star, EDGES_PER_HALF), np.int64)
        ids_lo = np.full((w_star, EDGES_PER_HALF), -1.0, np.float32)
        nd_lo = np.zeros((w_star, EDGES_PER_HALF), np.int64)  # src node ids
        idx_hi = np.zeros_like(idx_lo)
        ids_hi = np.full_like(ids_lo, -1.0)
        nd_hi = np.zeros_like(nd_lo)
        # edges are dst-sorted; window edge groups are contiguous
        wbounds = np.searchsorted(dl, [a for a, _ in core_windows[k]] + [npc])
        for w, (a, b) in enumerate(core_windows[k]):
            lo_m = is_lo[wbounds[w] : wbounds[w + 1]]
            e_dst = dl[wbounds[w] : wbounds[w + 1]]
            e_src = sv[wbounds[w] : wbounds[w + 1]]
            for half, m in ((0, lo_m), (1, ~lo_m)):
                r = row_of[e_src[m]] - (0 if half == 0 else half_rows)
                cnt = r.shape[0]
                assert cnt <= EDGES_PER_HALF
                tgt_idx = idx_lo if half == 0 else idx_hi
                tgt_ids = ids_lo if half == 0 else ids_hi
                tgt_nd = nd_lo if half == 0 else nd_hi
                tgt_idx[w, :cnt] = r
                tgt_ids[w, :cnt] = (e_dst[m] - a).astype(np.float32)
                tgt_nd[w, :cnt] = e_src[m]

        def _ids_wrap(ids_arr):
            # [n_chunks, 128] -> [128, n_chunks] bf16
            return np.ascontiguousarray(
                ids_arr.reshape(n_chunks, 128).T.astype(ml_dtypes.bfloat16)
            )

        def _xg(nd_arr):
            # pregathered layer-1 rows: [128, n_chunks * F] bf16
            g = x_bf[nd_arr.reshape(n_chunks, 128)]  # [nch, 128, F]
            return np.ascontiguousarray(
                g.transpose(1, 0, 2).reshape(128, n_chunks * F)
            )

        per_core.append(
            dict(
                idx_lo=_wrap_idx(idx_lo.reshape(-1)),
                idx_hi=_wrap_idx(idx_hi.reshape(-1)),
                ids_lo=_ids_wrap(ids_lo),
                ids_hi=_ids_wrap(ids_hi),
                xg_lo=_xg(nd_lo),
                xg_hi=_xg(nd_hi),
            )
        )

    iota_np = np.ascontiguousarray(
        np.tile(np.arange(128, dtype=np.float32), (128, 1)).astype(
            ml_dtypes.bfloat16
        )
    )

    in_maps = []
    for k in range(N_CORES):
        g = np.arange(k * npc, (k + 1) * npc)
        x_fm = np.zeros((F, ls), ml_dtypes.bfloat16)
        x_fm[:, slot[g]] = x_bf[g].T
        b_flat = np.full(ls, -1.0, np.float32)
        b_flat[slot[g]] = batch[g].astype(np.float32)
        batch_nm = b_flat.reshape(w_star, 128).T  # [p, c]
        b_onehot = (
            batch_nm[:, :, None] == np.arange(64, dtype=np.float32)[None, None, :]
        )
        b_onehot = np.ascontiguousarray(
            b_onehot.reshape(128, w_star * 64).astype(ml_dtypes.bfloat16)
        )
        m = dict(
            x_fm=x_fm,
            b_onehot=b_onehot,
            iota=iota_np,
            idx_lo=per_core[k]["idx_lo"],
            idx_hi=per_core[k]["idx_hi"],
            ids_lo=per_core[k]["ids_lo"],
            ids_hi=per_core[k]["ids_hi"],
            xg_lo=per_core[k]["xg_lo"],
            xg_hi=per_core[k]["xg_hi"],
            w1relT=np.ascontiguousarray(params["W1_rel"].T.astype(ml_dtypes.bfloat16)),
            w1rootT=np.ascontiguousarray(
                params["W1_root"].T.astype(ml_dtypes.bfloat16)
            ),
            w2relT=np.ascontiguousarray(params["W2_rel"].T.astype(ml_dtypes.bfloat16)),
            w2rootT=np.ascontiguousarray(
                params["W2_root"].T.astype(ml_dtypes.bfloat16)
            ),
            w3relT=np.ascontiguousarray(params["W3_rel"].T.astype(ml_dtypes.bfloat16)),
            w3rootT=np.ascontiguousarray(
                params["W3_root"].T.astype(ml_dtypes.bfloat16)
            ),
            b1=np.ascontiguousarray(params["b1_rel"].astype(np.float32).reshape(F, 1)),
            b2=np.ascontiguousarray(params["b2_rel"].astype(np.float32).reshape(F, 1)),
            b3=np.ascontiguousarray(params["b3_rel"].astype(np.float32).reshape(F, 1)),
            wlinT=np.ascontiguousarray(params["W_lin"].T.astype(np.float32)),
        )
        in_maps.append(m)

    meta = dict(
        w_star=w_star,
        ls=ls,
        rows=rows,
        half_rows=half_rows,
        n_graphs=n_graphs,
        mid=mid,
        n_chunks=n_chunks,
        n_ops=n_ops,
    )
    return meta, in_maps


def build_nc(meta, n_graphs_pad=64, debug_dumps=False):
    w_star = meta["w_star"]
    ls = meta["ls"]
    rows = meta["rows"]
    half_rows = meta["half_rows"]
    mid = meta["mid"]
    n_chunks = meta["n_chunks"]
    n_ops = meta["n_ops"]
    ng = n_graphs_pad
    gb = min(GB, n_ops)
    halves = ("lo", "hi")

    def nch_of(o):
        return min(n_chunks, (o + 1) * CPO) - o * CPO

    def q_of(h, o):
        return (o % 2) if h == "lo" else 2 + (o % 2)

    nc = bacc.Bacc(
        "TRN2",
        target_bir_lowering=False,
        debug=False,
        num_devices=N_CORES,
        num_swdge_queues=4,
    )

    # --- I/O ---
    x_fm_d = nc.dram_tensor("x_fm", [F, ls], BF16, kind="ExternalInput")
    bone_d = nc.dram_tensor("b_onehot", [128, w_star * 64], BF16, kind="ExternalInput")
    iota_d = nc.dram_tensor("iota", [128, 128], BF16, kind="ExternalInput")
    idx_d = {
        h: nc.dram_tensor(
            f"idx_{h}", [128, n_chunks * 8], I16, kind="ExternalInput"
        )
        for h in halves
    }
    ids_d = {
        h: nc.dram_tensor(f"ids_{h}", [128, n_chunks], BF16, kind="ExternalInput")
        for h in halves
    }
    xg_d = {
        h: nc.dram_tensor(
            f"xg_{h}", [128, n_chunks * F], BF16, kind="ExternalInput"
        )
        for h in halves
    }
    w_d = {}
    for l in (1, 2, 3):
        for p in ("rel", "root"):
            w_d[l, p] = nc.dram_tensor(f"w{l}{p}T", [F, F], BF16, kind="ExternalInput")
    b_d = {l: nc.dram_tensor(f"b{l}", [F, 1], F32, kind="ExternalInput") for l in (1, 2, 3)}
    wlin_d = nc.dram_tensor("wlinT", [F, N_CLASSES], F32, kind="ExternalInput")
    out_d = nc.dram_tensor("out_partial", [N_CLASSES, ng], F32, kind="ExternalOutput")
    dump_d = {}
    if debug_dumps:
        for l in (1, 2, 3):
            dump_d[l] = nc.dram_tensor(f"h_dump_{l}", [F, ls], BF16,
                                       kind="ExternalOutput")
        dump_d["agg1"] = nc.dram_tensor("agg_dump_1", [F, ls], BF16,
                                        kind="ExternalOutput")
        dump_d["g"] = nc.dram_tensor("g_dump", [128, CPO * F], BF16,
                                     kind="ExternalOutput")
        dump_d["hf1"] = nc.dram_tensor("hf1_dump", [rows, F], BF16,
                                       kind="ExternalOutput")

    relu = mybir.ActivationFunctionType.Relu
    ident = mybir.ActivationFunctionType.Identity
    copy_f = mybir.ActivationFunctionType.Copy

    with tile.TileContext(nc) as tc:
        with (
            tc.tile_pool(name="const", bufs=1) as constp,
            tc.tile_pool(name="state", bufs=1) as statep,
            tc.tile_pool(name="gpool", bufs=gb) as gpool,
            tc.tile_pool(name="spool", bufs=SB) as spool,
            tc.tile_pool(name="psa", bufs=4, space="PSUM") as psa,
            tc.tile_pool(name="psd", bufs=2, space="PSUM") as psd,
            tc.tile_pool(name="psp", bufs=1, space="PSUM") as psp,
            tc.tile_pool(name="dram", bufs=1, space="DRAM") as dramp,
        ):
            nc.gpsimd.load_library(library_config.mlp)
            # NOTE on prepare_only/trigger_dma: attempted, abandoned. Tile's
            # consumer sync for gen_mode=1 preps releases at desc-gen time
            # (IncSwdgeSem doorbells), and explicit consumer gating on the
            # prep's sem= still showed progressive staleness during burst
            # fires on HW (completion increments lead the data). gen_mode=0
            # gathers have correct Tile-managed completion semantics.

            # ---- constants, in consumption order per HWDGE lane ----
            # sync: iota/ids (DVE s-gen needs them at t~0), then the L1
            # xg_lo stream; idx tiles (needed only post-AG1) ride late.
            iota_t = constp.tile([128, 128], BF16)
            nc.sync.dma_start(iota_t[:], iota_d[:])
            ids_t = {}
            for h in halves:
                st = constp.tile([128, n_chunks], BF16, name=f"ids_{h}")
                nc.sync.dma_start(st[:], ids_d[h][:])
                ids_t[h] = st
            # scalar: x_fm + weights (needed at dense, ~mid-L1), then xg_hi
            x_fm_t = statep.tile([F, ls], BF16, tag="h0")
            nc.scalar.dma_start(x_fm_t[:], x_fm_d[:])
            w_t = {}
            for key, d in w_d.items():
                wt = constp.tile([F, F], BF16, name=f"w_{key[0]}_{key[1]}")
                nc.scalar.dma_start(wt[:], d[:])
                w_t[key] = wt
            b_t = {}
            for l, d in b_d.items():
                bt = constp.tile([F, 1], F32, name=f"b_{l}")
                nc.scalar.dma_start(bt[:], d[:])
                b_t[l] = bt
            wlin_t = constp.tile([F, N_CLASSES], F32)
            nc.scalar.dma_start(wlin_t[:], wlin_d[:])

            # ---- per-layer state ----
            h_fm = x_fm_t
            hf = {}       # AllGather outputs per layer
            ag_in = {}
            for layer in (1, 2):
                ag_in[layer] = dramp.tile(
                    [128, ls], BF16, name=f"agin_{layer}", tag=f"agin{layer % 2}"
                )
                hf[layer] = dramp.tile(
                    [rows, F], BF16, name=f"hf_{layer}", tag=f"hf{layer}",
                    addr_space="Shared",
                )

            def gather_src(layer, h):
                tbl = hf[layer - 1]
                return tbl[0:half_rows, :] if h == "lo" else tbl[half_rows:rows, :]

            def emit_sgen(layer, s_tiles, o):
                c0 = o * CPO
                nch = nch_of(o)
                for h in halves:
                    st_ = spool.tile(
                        [128, nch, 128], BF16,
                        name=f"s_{layer}_{h}_{o}", tag=f"s_{h}",
                        padded_shape=[128, CPO, 128],
                    )
                    in0 = iota_t[:].unsqueeze(1).broadcast_to([128, nch, 128])
                    in1 = ids_t[h][:, c0 : c0 + nch].unsqueeze(2).broadcast_to(
                        [128, nch, 128]
                    )
                    nc.vector.tensor_tensor(
                        st_[:], in0, in1, mybir.AluOpType.is_equal
                    )
                    s_tiles[h][o] = st_

            def alloc_gtile(layer, h, o):
                nch = nch_of(o)
                return gpool.tile(
                    [128, nch, F], BF16,
                    name=f"g_{layer}_{h}_{o}", tag=f"g_{h}",
                    padded_shape=[128, CPO, F],
                )

            def emit_l1_load(g_tiles, o):
                nch = nch_of(o)
                c0 = o * CPO
                for h, eng in (("lo", nc.sync), ("hi", nc.scalar)):
                    gt = alloc_gtile(1, h, o)
                    eng.dma_start(
                        gt[:].rearrange("p c f -> p (c f)"),
                        xg_d[h][:, c0 * F : (c0 + nch) * F],
                    )
                    g_tiles[h][o] = gt

            def emit_gather(layer, g_tiles, o):
                nch = nch_of(o)
                c0 = o * CPO
                nidx = nch * 128
                for h in halves:
                    gt = alloc_gtile(layer, h, o)
                    nc.gpsimd.dma_gather(
                        gt[:],
                        gather_src(layer, h),
                        idx_t[h][:, c0 * 8 : (c0 + nch) * 8],
                        nidx,
                        nidx,
                        F,
                        single_packet=True,
                        queue_num=q_of(h, o),
                    )
                    g_tiles[h][o] = gt

            def emit_layer_compute(layer, g_tiles, s_tiles, fence_src):
                """Emit windows + dense + stage for `layer`.

                fence_src: SBUF tile loaded from hf post-collective (layer>=2)
                anchoring the fence matmul that carries the first gather-op's
                completion wait in the PE stream.
                """
                agg_fm = statep.tile([F, ls], BF16, tag="agg", name=f"agg_{layer}")
                h_next = statep.tile(
                    [F, ls], BF16, tag=f"h{layer % 2}", name=f"h_{layer}"
                )
                h_nm = statep.tile(
                    [128, w_star, F], BF16, tag="hnm", name=f"hnm_{layer}"
                )

                # emission pointers for slot-release-paced late work
                next_sgen = [min(SB, n_ops)]
                next_gop = [gb]

                def release(w):
                    # s-gen for op o allowed once window consuming op o-SB done
                    while next_sgen[0] < n_ops and (
                        ((next_sgen[0] - SB) * CPO + CPO - 1) // K_LO <= w
                    ):
                        emit_sgen(layer, s_tiles, next_sgen[0])
                        next_sgen[0] += 1
                    while next_gop[0] < n_ops and (
                        ((next_gop[0] - gb) * CPO + CPO - 1) // K_LO <= w
                    ):
                        o = next_gop[0]
                        if layer == 1:
                            emit_l1_load(g_tiles, o)
                        else:
                            emit_gather(layer, g_tiles, o)
                        next_gop[0] += 1

                def dense_block(c0_, cq1):
                    while c0_ < cq1:
                        cw = min(512, cq1 - c0_)
                        ps = psd.tile(
                            [128, 512], F32, name=f"psd_{layer}_{c0_}", tag="psd"
                        )
                        sl2 = slice(c0_, c0_ + cw)
                        nc.tensor.matmul(
                            ps[:, :cw], w_t[layer, "rel"][:], agg_fm[:, sl2],
                            start=True, stop=False,
                        )
                        nc.tensor.matmul(
                            ps[:, :cw], w_t[layer, "root"][:], h_fm[:, sl2],
                            start=False, stop=True,
                        )
                        nc.scalar.activation(
                            h_next[:, sl2], ps[:, :cw],
                            relu if layer < 3 else ident,
                            bias=b_t[layer][:],
                        )
                        c0_ += cw

                def stage(a, b):
                    nc.sync.dma_start_transpose(
                        h_nm[:, a:b, :], h_next[:, a * 128 : b * 128]
                    )
                    if layer < 3:
                        nc.sync.dma_start(
                            ag_in[layer][:, a * F : b * F], h_nm[:, a:b, :]
                        )

                for w in range(w_star):
                    ps = psa.tile(
                        [128, 128], F32, name=f"psagg_{layer}_{w}", tag="psagg"
                    )
                    for j in range(CHUNKS_PER_WIN):
                        h = "lo" if j < K_LO else "hi"
                        cc = w * K_LO + (j % K_LO)
                        o, sl_ = cc // CPO, cc % CPO
                        nc.tensor.matmul(
                            ps[:],
                            g_tiles[h][o][:, sl_, :],
                            s_tiles[h][o][:, sl_, :],
                            start=(j == 0),
                            stop=(j == CHUNKS_PER_WIN - 1),
                        )
                    nc.scalar.activation(
                        agg_fm[:, w * 128 : (w + 1) * 128], ps[:], copy_f
                    )
                    if debug_dumps and layer == 2 and w == 0:
                        nc.sync.dma_start(
                            dump_d["g"][:],
                            g_tiles["lo"][0][:].rearrange("p c f -> p (c f)"),
                        )
                    release(w)
                    if w == mid - 1:
                        dense_block(0, mid * 128)
                        stage(0, mid)
                dense_block(mid * 128, ls)
                stage(mid, w_star)
                if debug_dumps:
                    nc.sync.dma_start(dump_d[layer][:], h_next[:])
                    if layer == 1:
                        nc.sync.dma_start(dump_d["agg1"][:], agg_fm[:])
                return h_next, h_nm

            # ================= emission =================
            # Layer 1: initial s-gen + loads
            s_tiles = {1: {h: [None] * n_ops for h in halves}}
            g_tiles = {1: {h: [None] * n_ops for h in halves}}
            for o in range(min(SB, n_ops)):
                emit_sgen(1, s_tiles[1], o)
            for o in range(gb):
                emit_l1_load(g_tiles[1], o)
            # idx tiles (gather metadata, first needed post-AG1) ride sync
            # behind the initial xg_lo burst
            idx_t = {}
            for h in halves:
                it = constp.tile([128, n_chunks * 8], I16, name=f"idx_{h}")
                nc.sync.dma_start(it[:], idx_d[h][:])
                idx_t[h] = it

            h1, _ = emit_layer_compute(1, g_tiles[1], s_tiles[1], None)

            # AG1, then layer-2 gathers (each waits the collective via the
            # Tile-tracked hf data dep; first gather head-blocks gpsimd
            # until AG1 completes, which is why AG1 is emitted first)
            s_tiles[2] = {h: [None] * n_ops for h in halves}
            g_tiles[2] = {h: [None] * n_ops for h in halves}
            nc.gpsimd.collective_compute(
                "AllGather",
                mybir.AluOpType.bypass,
                replica_groups=[list(range(N_CORES))],
                ins=[ag_in[1][:]],
                outs=[hf[1][:]],
            )
            if debug_dumps:
                nc.sync.dma_start(dump_d["hf1"][:], hf[1][:])
            for o in range(gb):
                emit_gather(2, g_tiles[2], o)

            # Layer 2 s-gen head + compute
            h_fm = h1
            for o in range(min(SB, n_ops)):
                emit_sgen(2, s_tiles[2], o)
            h2, _ = emit_layer_compute(2, g_tiles[2], s_tiles[2], None)

            # AG2, then layer-3 gathers
            s_tiles[3] = {h: [None] * n_ops for h in halves}
            g_tiles[3] = {h: [None] * n_ops for h in halves}
            nc.gpsimd.collective_compute(
                "AllGather",
                mybir.AluOpType.bypass,
                replica_groups=[list(range(N_CORES))],
                ins=[ag_in[2][:]],
                outs=[hf[2][:]],
            )
            for o in range(gb):
                emit_gather(3, g_tiles[3], o)
            # bone (pooling one-hot) needed only at the very end
            bone_t = constp.tile([128, w_star * 64], BF16)
            nc.scalar.dma_start(bone_t[:], bone_d[:])

            # Layer 3 s-gen head + compute
            h_fm = h2
            for o in range(min(SB, n_ops)):
                emit_sgen(3, s_tiles[3], o)
            _, h_nm3 = emit_layer_compute(3, g_tiles[3], s_tiles[3], None)

            # pooling: pooledT[f, g] += h_nm[:, c, :].T @ B
            ps_pool = psp.tile([128, ng], F32, tag="pspool")
            for c in range(w_star):
                nc.tensor.matmul(
                    ps_pool[:],
                    h_nm3[:, c, :],
                    bone_t[:, c * ng : (c + 1) * ng],
                    start=(c == 0),
                    stop=(c == w_star - 1),
                )
            pooledT = statep.tile([128, ng], F32, tag="pooledT")
            nc.scalar.activation(pooledT[:], ps_pool[:], copy_f)
            ps_head = psp.tile([N_CLASSES, ng], F32, tag="pshead")
            nc.tensor.matmul(ps_head[:], wlin_t[:], pooledT[:])
            out_sb = statep.tile([N_CLASSES, ng], F32, tag="outsb")
            nc.vector.tensor_copy(out_sb[:], ps_head[:])
            nc.sync.dma_start(out_d[:], out_sb[:])

    nc.compile()
    return nc


def postprocess(results, batch, b_lin, n_graphs):
    """results: list of per-core dicts with 'out_partial' [10, ng]."""
    total = np.zeros_like(np.asarray(results[0]["out_partial"], np.float32))
    for r in results:
        total += np.asarray(r["out_partial"], np.float32)
    cnt = np.bincount(np.asarray(batch, np.int64), minlength=n_graphs).astype(
        np.float32
    )
    cnt = np.maximum(cnt, 1.0)
    logits = total[:, :n_graphs].T / cnt[:, None] + np.asarray(b_lin, np.float32)[None, :]
    return logits.astype(np.float32)


# ----------------------------------------------------------------------------
# harness entry point
# ----------------------------------------------------------------------------
from concourse.bass_utils import run_bass_kernel_spmd

_CACHE = {}


def kernel(x, edge_index, batch,
           W1_rel, b1_rel, W1_root,
           W2_rel, b2_rel, W2_root,
           W3_rel, b3_rel, W3_root,
           W_lin, b_lin):
    params = dict(W1_rel=W1_rel, b1_rel=b1_rel, W1_root=W1_root,
                  W2_rel=W2_rel, b2_rel=b2_rel, W2_root=W2_root,
                  W3_rel=W3_rel, b3_rel=b3_rel, W3_root=W3_root,
                  W_lin=W_lin, b_lin=b_lin)
    n_nodes = int(np.asarray(x).shape[0])
    n_graphs = 64
    meta, in_maps = preprocess(x, edge_index, batch, params, n_nodes, n_graphs)
    key = (meta["w_star"], meta["n_ops"])
    if key not in _CACHE:
        _CACHE[key] = build_nc(meta)
    nc = _CACHE[key]
    res = run_bass_kernel_spmd(nc, in_maps, core_ids=list(range(N_CORES)))
    return postprocess(res.results, batch, b_lin, n_graphs)
